# revision 1
# baseline (speedup 1.0000x reference)
"""Trainium2 Bass kernel for nn_Encoder (VGAE-style GNN encoder).

Computation (see reference):
  deg/norms from src/dst; h = relu(norm_dst * segsum_dst((feat*norm_src @ W1)[src]) + b1)
  agg2 = segsum_dst(h[src]);  mu = (agg2*norm_dst) @ W_mu + b_mu ; log_sigma likewise
  z = mu + noise * exp(log_sigma)

Strategy (graph/data parallel, dst-sharded, per the sharding hint):
  - nodes are padded to NPAD and sharded SHARD per core; edges assigned to the
    core owning their dst node.
  - host does index preprocessing only: degree bincount -> norm scalars,
    edge sort by (dst-supertile, src-window, dst), 128-padded groups,
    int16 gather index lists, per-block one-hot positions.
  - device does all feature-space work:
      phase1: x1 = (feat*norm_src) @ W1 per shard -> AllGather fp16 table
      round1: dma_gather x1[src] (256B rows) ; per 128-edge block build a
              one-hot [e,dst_local] with a DVE is_equal and matmul-accumulate
              the segment sum in PSUM ; relu epilogue -> h shard -> AllGather
      round2: same gather/scatter from h ; epilogue: transpose, W_mu/W_sig
              matmuls, exp on ACT, z = mu + noise*exp(ls), transpose back.
"""

import sys
import os
import numpy as np
from contextlib import ExitStack

if "/opt/trn_rl_repo" not in sys.path:
    sys.path.insert(0, "/opt/trn_rl_repo")

import concourse.bass as bass
import concourse.mybir as mybir
import concourse.tile as tile
from concourse.bacc import Bacc
from concourse.bass_utils import run_bass_kernel_spmd

F16 = mybir.dt.float16
F32 = mybir.dt.float32
I16 = mybir.dt.int16
ALU = mybir.AluOpType
ACTF = mybir.ActivationFunctionType

ST = 128  # supertile = dst nodes per PSUM accumulation tile


def default_cfg(n, e, f, h):
    ncore = 8
    shard = -(-n // (ncore * ST)) * ST  # ceil to multiple of 128
    npad = shard * ncore
    nwin = 4
    win = -(-npad // nwin)
    assert win <= 32768, "int16 gather index range"
    nst = shard // ST
    # supertiles per gather group: largest divisor of nst keeping gathers
    # comfortably under the ~12800-idx SWDGE ring ceiling
    sb = 1
    for cand in range(1, nst + 1):
        if nst % cand == 0 and cand * 8 * 128 <= int(os.environ.get("KSBCAP", "2048")):
            sb = cand
    return dict(N=n, E=e, F=f, H=h, NCORE=ncore, SHARD=shard, NPAD=npad,
                NWIN=nwin, WIN=win, NST=nst, SB=sb)


def build_plan(src, dst, cfg):
    """Host-side index preprocessing. Returns per-core gather/one-hot arrays."""
    N, NCORE = cfg["N"], cfg["NCORE"]
    SHARD, NWIN, WIN, NST, SB = (cfg[k] for k in ("SHARD", "NWIN", "WIN", "NST", "SB"))
    src = np.asarray(src).astype(np.int64)
    dst = np.asarray(dst).astype(np.int64)

    core_of = dst // SHARD
    per_core = []
    cblk_need = 1
    for c in range(NCORE):
        sel = core_of == c
        s_c, d_c = src[sel], dst[sel]
        s_local = (d_c - c * SHARD) // ST
        w = s_c // WIN
        order = np.lexsort((d_c, w, s_local))
        s_c, d_c, s_local, w = s_c[order], d_c[order], s_local[order], w[order]
        gid = s_local * NWIN + w
        cnt = np.bincount(gid, minlength=NST * NWIN)
        cblk_need = max(cblk_need, int(-(-cnt.max() // ST)))
        per_core.append((s_c, d_c, gid, cnt, c))
    CBLK = int(cblk_need)
    GLEN = CBLK * ST                      # padded edges per (supertile, window) group
    NIDX = SB * CBLK * ST                 # idxs per gather instruction
    NCOLS = NIDX // 16                    # int16 idx columns per gather
    NG = NST // SB                        # gather groups per round
    NBLK = NST * NWIN * CBLK              # one-hot blocks per round

    plans = []
    for (s_c, d_c, gid, cnt, c) in per_core:
        idx_flat = np.zeros(NST * NWIN * GLEN, dtype=np.int16)
        dloc_flat = np.full(NST * NWIN * GLEN, 300.0, dtype=np.float32)
        starts = np.concatenate(([0], np.cumsum(cnt)))
        # position of each edge inside the padded group layout
        pos = np.arange(len(s_c)) - starts[gid] + gid * GLEN
        idx_flat[pos] = (s_c % WIN).astype(np.int16)
        dloc_flat[pos] = (d_c - (c * SHARD + (gid // NWIN) * ST)).astype(np.float32)
        # eidx: per gather (g, w): concat si groups; wrap 16. The final
        # group's trailing pad slots become -1 (ucode skips their
        # descriptors); gcnt carries the per-gather valid count.
        eidx = np.zeros((128, NG * NWIN * NCOLS), dtype=np.int16)
        gcnt = np.zeros((1, NG * NWIN), dtype=np.int32)
        by_group = idx_flat.reshape(NST, NWIN, GLEN)
        cnt2 = cnt.reshape(NST, NWIN)
        for g in range(NG):
            for w in range(NWIN):
                lst = by_group[g * SB:(g + 1) * SB, w, :].reshape(-1).copy()
                last_s = g * SB + SB - 1
                n_tail = GLEN - int(cnt2[last_s, w])
                if n_tail > 0:
                    lst[NIDX - n_tail:] = -1
                gcnt[0, g * NWIN + w] = NIDX - n_tail
                wrapped = lst.reshape(NCOLS, 16).T  # [16, NCOLS]
                col0 = (g * NWIN + w) * NCOLS
                eidx[:, col0:col0 + NCOLS] = np.tile(wrapped, (8, 1))
        # dstloc: col (s, w, k) ; partition p = edge k*128+p of group (s, w)
        dstloc = dloc_flat.reshape(NST * NWIN * CBLK, 128).T.copy()  # [128, NBLK]
        plans.append(dict(eidx=eidx, dstloc=dstloc, gcnt=gcnt))
    meta = dict(CBLK=CBLK, NIDX=NIDX, NCOLS=NCOLS, NG=NG, NBLK=NBLK)
    return plans, meta


def build_program(cfg, meta, sim_mode=False):
    NCORE, SHARD, NPAD = cfg["NCORE"], cfg["SHARD"], cfg["NPAD"]
    NWIN, WIN, NST, SB, F, H = (cfg[k] for k in ("NWIN", "WIN", "NST", "SB", "F", "H"))
    CBLK, NIDX, NCOLS, NG, NBLK = (meta[k] for k in ("CBLK", "NIDX", "NCOLS", "NG", "NBLK"))

    nc = Bacc(trn_type="TRN2", num_devices=NCORE)

    feat_shard = nc.dram_tensor("feat_shard", [SHARD, F], F32, kind="ExternalInput")
    nsrc = nc.dram_tensor("nsrc", [128, NST], F32, kind="ExternalInput")
    ndst = nc.dram_tensor("ndst", [128, NST], F32, kind="ExternalInput")
    w1_16 = nc.dram_tensor("w1_16", [F, H], F16, kind="ExternalInput")
    wmu_16 = nc.dram_tensor("wmu_16", [H, H], F16, kind="ExternalInput")
    wsig_16 = nc.dram_tensor("wsig_16", [H, H], F16, kind="ExternalInput")
    b1_rep = nc.dram_tensor("b1_rep", [128, H], F32, kind="ExternalInput")
    bmu_col = nc.dram_tensor("bmu_col", [H, 1], F32, kind="ExternalInput")
    bsig_col = nc.dram_tensor("bsig_col", [H, 1], F32, kind="ExternalInput")
    eye16_d = nc.dram_tensor("eye16", [128, 128], F16, kind="ExternalInput")
    eye32_d = nc.dram_tensor("eye32", [H, H], F32, kind="ExternalInput")
    iota16_d = nc.dram_tensor("iota16", [128, 128], F16, kind="ExternalInput")
    eidx_d = nc.dram_tensor("eidx", [128, NG * NWIN * NCOLS], I16, kind="ExternalInput")
    dstloc_d = nc.dram_tensor("dstloc", [128, NBLK], F32, kind="ExternalInput")
    gcnt_d = nc.dram_tensor("gcnt", [1, NG * NWIN], mybir.dt.int32,
                            kind="ExternalInput")
    noise_t = nc.dram_tensor("noise_t", [H, SHARD], F32, kind="ExternalInput")
    z_out = nc.dram_tensor("z_out", [SHARD, H], F32, kind="ExternalOutput")
    dbg = bool(int(os.environ.get("KDBG", "0")))
    if dbg:
        x1_dbg = nc.dram_tensor("x1_dbg", [SHARD, 128], F16, kind="ExternalOutput")
        h_dbg = nc.dram_tensor("h_dbg", [SHARD, 128], F16, kind="ExternalOutput")
        agg_dbg = nc.dram_tensor("agg_dbg", [SHARD, H], F32, kind="ExternalOutput")
        a2s_dbg = nc.dram_tensor("a2s_dbg", [SHARD, H], F16, kind="ExternalOutput")
        mu_dbg = nc.dram_tensor("mu_dbg", [H, SHARD], F32, kind="ExternalOutput")
        es_dbg = nc.dram_tensor("es_dbg", [H, SHARD], F32, kind="ExternalOutput")

    x1_shard = nc.dram_tensor("x1_shard", [SHARD, 128], F16, kind="Internal")
    h_shard = nc.dram_tensor("h_shard", [SHARD, 128], F16, kind="Internal")
    x1_table = nc.dram_tensor("x1_table", [NPAD, 128], F16, kind="Internal",
                              addr_space="Shared")
    h_table = nc.dram_tensor("h_table", [NPAD, 128], F16, kind="Internal",
                             addr_space="Shared")
    groups = [list(range(NCORE))]

    with tile.TileContext(nc) as tc, ExitStack() as ctx:
        consts = ctx.enter_context(tc.tile_pool(name="consts", bufs=1))

        def cload(dram, shape, dtype, tag):
            t = consts.tile(shape, dtype, tag=tag)
            nc.sync.dma_start(t[:], dram[:])
            return t

        w1_sb = cload(w1_16, [F, H], F16, "w1")
        wmu_sb = cload(wmu_16, [H, H], F16, "wmu")
        wsig_sb = cload(wsig_16, [H, H], F16, "wsig")
        nsrc_sb = cload(nsrc, [128, NST], F32, "nsrc")
        ndst_sb = cload(ndst, [128, NST], F32, "ndst")
        b1_sb = cload(b1_rep, [128, H], F32, "b1")
        bmu_sb = cload(bmu_col, [H, 1], F32, "bmu")
        bsig_sb = cload(bsig_col, [H, 1], F32, "bsig")
        eye16 = cload(eye16_d, [128, 128], F16, "eye16")
        eye32 = cload(eye32_d, [H, H], F32, "eye32")
        iota16 = cload(iota16_d, [128, 128], F16, "iota16")
        eidx_sb = cload(eidx_d, [128, NG * NWIN * NCOLS], I16, "eidx")
        gcnt_sb = cload(gcnt_d, [1, NG * NWIN], mybir.dt.int32, "gcnt")
        dstloc_sb = cload(dstloc_d, [128, NBLK], F32, "dstloc")

        # ---------------- phase 1: x1 = (feat*nsrc) @ W1 on own shard -------
        with tc.tile_pool(name="p1", bufs=3) as p1, \
             tc.tile_pool(name="p1ps", bufs=2, space="PSUM") as p1ps:
            for t in range(NST):
                ft = p1.tile([128, F], F32, tag="ft")
                nc.sync.dma_start(ft[:], feat_shard[t * 128:(t + 1) * 128, :])
                fsc = p1.tile([128, F], F16, tag="fsc")
                nc.vector.tensor_scalar(fsc[:], ft[:], nsrc_sb[:, t:t + 1], None,
                                        ALU.mult)
                ftp = p1ps.tile([F, 128], F16, tag="ftp")
                nc.tensor.matmul(ftp[:], fsc[:], eye16[:], is_transpose=True)
                fts = p1.tile([F, 128], F16, tag="fts")
                nc.vector.tensor_copy(fts[:], ftp[:])
                x1p = p1ps.tile([128, H], F32, tag="x1p")
                nc.tensor.matmul(x1p[:], fts[:], w1_sb[:], start=True, stop=True)
                xst = p1.tile([128, 128], F16, tag="xst")
                nc.vector.tensor_copy(xst[:, 0:H], x1p[:])
                nc.sync.dma_start(x1_shard[t * 128:(t + 1) * 128, :], xst[:])
                if dbg:
                    nc.sync.dma_start(x1_dbg[t * 128:(t + 1) * 128, :], xst[:])

        if sim_mode:
            nc.sync.dma_start(x1_table[0:SHARD, :], x1_shard[:])
        else:
            nc.gpsimd.collective_compute("AllGather", ALU.bypass, groups,
                                         ins=[x1_shard[:]], outs=[x1_table[:]])

        # ---------------- message-passing round ----------------------------
        def mp_round(table, epilogue, ng_limit=None, cregs=[]):
            if not cregs:
                cregs.extend(nc.gpsimd.alloc_register(f"gcnt_r{i}")
                             for i in range(8))
            with tc.tile_pool(name="msgs", bufs=2) as msgs, \
                 tc.tile_pool(name="ohp", bufs=4) as ohp, \
                 tc.tile_pool(name="aggps", bufs=2, space="PSUM") as aggps, \
                 tc.tile_pool(name="epi", bufs=3) as epi, \
                 tc.tile_pool(name="episb", bufs=3) as episb, \
                 tc.tile_pool(name="epips", bufs=1, space="PSUM") as epips, \
                 tc.tile_pool(name="epips2", bufs=1, space="PSUM") as epips2:
                for g in range(NG if ng_limit is None else min(ng_limit, NG)):
                    mt = []
                    for w in range(NWIN):
                        m = msgs.tile([128, SB * CBLK, 128], F16, tag=f"m{w}")
                        col0 = (g * NWIN + w) * NCOLS
                        gi = g * NWIN + w
                        creg = cregs[gi % 8]
                        nc.gpsimd.reg_load(creg, gcnt_sb[0:1, gi:gi + 1])
                        nc.gpsimd.dma_gather(
                            m[:], table[w * WIN:(w + 1) * WIN, :],
                            eidx_sb[:, col0:col0 + NCOLS],
                            num_idxs=NIDX, num_idxs_reg=creg, elem_size=128,
                            single_packet=False)
                        mt.append(m)
                    for si in range(SB):
                        s = g * SB + si
                        agg = aggps.tile([128, H], F32, tag="agg")
                        for w in range(NWIN):
                            for k in range(CBLK):
                                col = (s * NWIN + w) * CBLK + k
                                oh = ohp.tile([128, 128], F16, tag="oh")
                                nc.vector.tensor_scalar(
                                    oh[:], iota16[:], dstloc_sb[:, col:col + 1],
                                    None, ALU.is_equal)
                                nc.tensor.matmul(
                                    agg[:], oh[:], mt[w][:, si * CBLK + k, 0:H],
                                    start=(w == 0 and k == 0),
                                    stop=(w == NWIN - 1 and k == CBLK - 1))
                        epilogue(s, agg, epi, episb, epips, epips2)

        def epi_round1(s, agg, epi, episb, epips, epips2):
            if dbg:
                ad = epi.tile([128, H], F32, tag="ad")
                nc.vector.tensor_copy(ad[:], agg[:])
                nc.sync.dma_start(agg_dbg[s * 128:(s + 1) * 128, :], ad[:])
            hp = epi.tile([128, H], F32, tag="hp")
            nc.vector.scalar_tensor_tensor(hp[:], agg[:], ndst_sb[:, s:s + 1],
                                           b1_sb[:], ALU.mult, ALU.add)
            hst = episb.tile([128, 128], F16, tag="hst")
            nc.scalar.activation(hst[:, 0:H], hp[:], ACTF.Relu,
                                 scale=nsrc_sb[:, s:s + 1])
            nc.sync.dma_start(h_shard[s * 128:(s + 1) * 128, :], hst[:])
            if dbg:
                nc.sync.dma_start(h_dbg[s * 128:(s + 1) * 128, :], hst[:])

        def epi_round2(s, agg, epi, episb, epips, epips2):
            a2s = epi.tile([128, H], F16, tag="a2s")
            nc.vector.tensor_scalar(a2s[:], agg[:], ndst_sb[:, s:s + 1], None,
                                    ALU.mult)
            if dbg:
                nc.sync.dma_start(a2s_dbg[s * 128:(s + 1) * 128, :], a2s[:])
            a2tp = epips.tile([H, 128], F16, tag="a2tp")
            nc.tensor.matmul(a2tp[:], a2s[:], eye16[:], is_transpose=True)
            a2t = epi.tile([H, 128], F16, tag="a2t")
            nc.vector.tensor_copy(a2t[:], a2tp[:])
            mup = epips2.tile([H, 128], F32, tag="mup")
            nc.tensor.matmul(mup[:], wmu_sb[:], a2t[:], start=True, stop=True)
            sgp = epips2.tile([H, 128], F32, tag="sgp")
            nc.tensor.matmul(sgp[:], wsig_sb[:], a2t[:], start=True, stop=True)
            mub = episb.tile([H, 128], F32, tag="mub")
            nc.scalar.activation(mub[:], mup[:], ACTF.Identity, bias=bmu_sb[:])
            es = episb.tile([H, 128], F32, tag="es")
            nc.scalar.activation(es[:], sgp[:], ACTF.Exp, bias=bsig_sb[:])
            if dbg:
                nc.sync.dma_start(mu_dbg[:, s * 128:(s + 1) * 128], mub[:])
                nc.sync.dma_start(es_dbg[:, s * 128:(s + 1) * 128], es[:])
            nzt = epi.tile([H, 128], F32, tag="nzt")
            nc.sync.dma_start(nzt[:], noise_t[:, s * 128:(s + 1) * 128])
            nz = episb.tile([H, 128], F32, tag="nz")
            nc.vector.scalar_tensor_tensor(nz[:], nzt[:], 1.0, es[:],
                                           ALU.mult, ALU.mult)
            zt = epi.tile([H, 128], F32, tag="zt")
            nc.vector.scalar_tensor_tensor(zt[:], mub[:], 0.0, nz[:],
                                           ALU.add, ALU.add)
            ztp = epips.tile([128, H], F32, tag="ztp")
            nc.tensor.matmul(ztp[:], zt[:], eye32[:], is_transpose=True)
            zst = episb.tile([128, H], F32, tag="zst")
            nc.vector.tensor_copy(zst[:], ztp[:])
            nc.sync.dma_start(z_out[s * 128:(s + 1) * 128, :], zst[:])

        kphase = int(os.environ.get("KPHASE", "4"))
        if kphase >= 2:
            mp_round(x1_table, epi_round1)
        if kphase >= 3:
            if sim_mode:
                nc.sync.dma_start(h_table[0:SHARD, :], h_shard[:])
            else:
                nc.gpsimd.collective_compute("AllGather", ALU.bypass, groups,
                                             ins=[h_shard[:]], outs=[h_table[:]])
        if kphase >= 4:
            mp_round(h_table, epi_round2,
                     ng_limit=int(os.environ.get("KR2G", str(NG))))

    nc.finalize()
    return nc


def host_inputs(feat, src, dst, noise, W1, b1, W_mu, b_mu, W_sig, b_sig,
                cfg, plans):
    N, NCORE, SHARD, NPAD = (cfg[k] for k in ("N", "NCORE", "SHARD", "NPAD"))
    NST, F, H = cfg["NST"], cfg["F"], cfg["H"]
    feat = np.asarray(feat, dtype=np.float32)
    noise = np.asarray(noise, dtype=np.float32)
    src = np.asarray(src); dst = np.asarray(dst)

    deg_out = np.bincount(src, minlength=NPAD).astype(np.float32)
    deg_in = np.bincount(dst, minlength=NPAD).astype(np.float32)
    norm_src = np.maximum(deg_out, 1.0) ** -0.5
    norm_dst = np.maximum(deg_in, 1.0) ** -0.5

    featp = np.zeros((NPAD, F), dtype=np.float32)
    featp[:N] = feat
    noisep = np.zeros((NPAD, H), dtype=np.float32)
    noisep[:N] = noise

    eye16 = np.eye(128, dtype=np.float16)
    eye32 = np.eye(H, dtype=np.float32)
    iota16 = np.tile(np.arange(128, dtype=np.float16)[None, :], (128, 1))
    shared = dict(
        w1_16=np.asarray(W1, dtype=np.float16),
        wmu_16=np.asarray(W_mu, dtype=np.float16),
        wsig_16=np.asarray(W_sig, dtype=np.float16),
        b1_rep=np.tile(np.asarray(b1, dtype=np.float32)[None, :], (128, 1)),
        bmu_col=np.asarray(b_mu, dtype=np.float32).reshape(H, 1),
        bsig_col=np.asarray(b_sig, dtype=np.float32).reshape(H, 1),
        eye16=eye16, eye32=eye32, iota16=iota16,
    )
    in_maps = []
    for c in range(NCORE):
        lo, hi = c * SHARD, (c + 1) * SHARD
        m = dict(shared)
        m["feat_shard"] = featp[lo:hi]
        m["nsrc"] = norm_src[lo:hi].reshape(NST, 128).T.copy()
        m["ndst"] = norm_dst[lo:hi].reshape(NST, 128).T.copy()
        m["noise_t"] = noisep[lo:hi].T.copy()
        m["eidx"] = plans[c]["eidx"]
        m["gcnt"] = plans[c]["gcnt"]
        m["dstloc"] = plans[c]["dstloc"]
        in_maps.append(m)
    return in_maps


def run(feat, src, dst, noise, W1, b1, W_mu, b_mu, W_sig, b_sig,
        cfg=None, **spmd_kwargs):
    if cfg is None:
        cfg = default_cfg(feat.shape[0], src.shape[0], feat.shape[1],
                          W1.shape[1])
    plans, meta = build_plan(src, dst, cfg)
    nc = build_program(cfg, meta)
    in_maps = host_inputs(feat, src, dst, noise, W1, b1, W_mu, b_mu,
                          W_sig, b_sig, cfg, plans)
    import time as _time
    last_exc = None
    for attempt in range(3):
        try:
            res = run_bass_kernel_spmd(nc, in_maps,
                                       core_ids=list(range(cfg["NCORE"])),
                                       **spmd_kwargs)
            break
        except Exception as e:  # transient NRT device errors: retry
            last_exc = e
            _time.sleep(10.0)
    else:
        raise last_exc
    z = np.concatenate([r["z_out"] for r in res.results], axis=0)[:cfg["N"]]
    return z.astype(np.float32), res


def kernel(feat, src, dst, noise, W1, b1, W_mu, b_mu, W_sig, b_sig):
    z, _ = run(feat, src, dst, noise, W1, b1, W_mu, b_mu, W_sig, b_sig)
    return z

